# revision 39
# baseline (speedup 1.0000x reference)
"""Trainium2 Bass kernel for nn_Attention_FRN (sparse windowed attention argmax).

Math: reference computes
    q  = (HSI flat -> (B,L,C)) @ Wq          (Wq = W_qkv1[:, :C])
    k  = (MSI flat -> (B,9L,C)) @ Wk         (Wk = W_qkv2[:, C:2C])
    attn[b,l,n] = <q[b,l], k[b,9l+n]> * scale
    out = argmax_n softmax(attn)  -> (n//3-1, n%3-1) offsets, reshaped (B,H,H,2)

Softmax and the positive scale are argmax-invariant, so both are skipped.
x and y inputs are unused (only x's shape matters).  The big k projection is
eliminated algebraically:  <q[l], mp[s] @ Wk> = <(q @ Wk^T)[l], mp[s]>, so we
only project the small hp tensor:  qm = (hp @ Wq) @ Wk^T, then
attn[l, n] = <qm[l], mp[9l+n]>.

Device mapping (DMA-roofline shaped; kernel is ~70%% mp-stream time):
  * mp and hp stream in fp8e4 (4.7MB + 0.5MB per core); M = Wq @ Wk^T is
    host-folded (weight constant folding) and ships fp16 (128KB).  The PE
    matmul allows mixed dtypes, so qm stays fp16 against fp8 mp.
  * Banded attn is computed windows-on-partitions: stationary = mp window
    (126 cols = 14 pixels x 9 offsets), moving = 14 qm pixel columns, so
    the cost-model matmul time is 2 moving rows per pixel instead of 18.
    Each pack of 126 pixels = 9 groups accumulated into one PSUM tile
    [128,126] on top of an affine-select-generated band mask (NEG off-band,
    added via identity matmul), then ACT-copied to SBUF fp16, PE-transposed
    to [126,128] (pixel partitions); DVE max/max_index give per-pixel top-8
    values + argmax position, shipped raw (host does the trivial index ->
    (dy,dx) decode).
  * The 5 tail packs (fed by the last 3 mp tiles) skip transpose/max: their
    masked fp16 attn tiles ship whole and the host takes the 9-value band
    argmax, cutting the post-stream critical chain to matmul -> copy -> DMA.
  * Top-2 gap ships with every pixel; gaps < GAP_TAU (~4 sigma of the fp8
    quantization noise) are exactly re-resolved on host in float64.
  * DMA structure: hp, then M, then 9 mp tiles sized/ordered so the last
    DMA feeds only the single last-emitted pack; qmT is produced in 512-
    column blocks interleaved with the packs they feed.

Sharding: 8 cores = B(4) x L-half(2).  Per core: hp shard (256, 2048),
mp shard (256, 18432), replicated weights + small constants.
"""

import numpy as np

B, C, H = 4, 256, 64
L = H * H                  # 4096 pixels per batch
L_SH = L // 2              # 2048 per core
S_SH = 9 * L_SH            # 18432 mp columns per core
PACK = 126                 # pixels per pack (9 groups of 14)
NPACKS = 17                # 16 full packs + one 32-pixel tail
NEG = -60000.0             # off-band mask value; fp16-safe

# mp tiles: (first_pack, n_packs, width_cols).  Issue order puts tile 8
# (packs 15, 16) early and tile 7 (single pack 14) last, so the final DMAs
# gate only three packs' chains.
MP_TILES = [(2 * t, 2, 2268) for t in range(7)] + [(14, 1, 1134), (15, 2, 1422)]
MP_ISSUE = [0, 1, 2, 3, 4, 5, 6, 8, 7]
# Pack emission is interleaved with the qmT l-blocks (512 qm columns each)
# that cover them, so the pack pipeline starts as soon as its block lands.
# The final segment is emitted matmuls-first (see _emit) so the PE queue
# never waits on an ACT copy between consecutive packs' matmuls.
QM_SEGMENTS = [(0, [0, 1, 2, 3]), (1, [4, 5, 6, 7]), (2, [8, 9, 10, 11])]
TAIL_PACKS = [12, 13, 14, 15, 16]
TAIL_A = [12, 13, 15, 16]   # ship after tile 8 lands; pack 14 goes alone last
TAIL_B = [14]

# the constants blob carries only M = Wq @ Wk^T (host-folded weights);
# identity and band mask are generated on-device with affine_select
CST_W = 512

_CACHE = {}


def _build_nc():
    import concourse.bacc as bacc
    import concourse.tile as tile
    from concourse import mybir

    f32 = mybir.dt.float32
    f16 = mybir.dt.float16
    f8 = mybir.dt.float8e4

    nc = bacc.Bacc(
        "TRN2",
        target_bir_lowering=False,
        debug=False,
        enable_asserts=False,
        num_devices=8,
    )
    mp_d = nc.dram_tensor("mp", [128, 2, S_SH], f8, kind="ExternalInput").ap()
    hp_d = nc.dram_tensor("hp", [128, 2, L_SH], f8, kind="ExternalInput").ap()
    cst_d = nc.dram_tensor("cst", [128, CST_W], f16, kind="ExternalInput").ap()
    out_d = nc.dram_tensor("outo", [128, NPACKS, 3], f16, kind="ExternalOutput").ap()
    zta_d = nc.dram_tensor("ztaila", [128, len(TAIL_A) * 128], f16,
                           kind="ExternalOutput").ap()
    ztb_d = nc.dram_tensor("ztailb", [128, len(TAIL_B) * 128], f16,
                           kind="ExternalOutput").ap()

    with tile.TileContext(nc) as tc:
        _emit(tc, out_d, zta_d, ztb_d, mp_d, hp_d, cst_d)
    nc.compile()
    return nc


def _emit(tc, out_d, zta_d, ztb_d, mp_d, hp_d, cst_d):
    from contextlib import ExitStack

    from concourse import mybir

    nc = tc.nc
    f32 = mybir.dt.float32
    f16 = mybir.dt.float16
    f8 = mybir.dt.float8e4
    u16 = mybir.dt.uint16
    dma = nc.sync.dma_start          # SP HWDGE ring: consts + hp + mp stream,
    #                                  then outputs (SP idles once mp issued;
    #                                  an ACT-ring output DMA would block the
    #                                  tail packs' PSUM->SBUF copies)

    with ExitStack() as ctx:
        consts = ctx.enter_context(tc.tile_pool(name="consts", bufs=1))
        mpp = ctx.enter_context(tc.tile_pool(name="mpp", bufs=len(MP_TILES)))
        ztp = ctx.enter_context(tc.tile_pool(name="ztp", bufs=3))

        hp_sb = consts.tile([128, 2, L_SH], f8)   # (c1%128, c1//128, l)
        m_sb = consts.tile([128, 2, C], f16)      # (c1%128, c1//128, c2)
        qmT_sb = consts.tile([128, 2, L_SH], f16)  # (c2%128, c2//128, l)
        m8buf = consts.tile([128, NPACKS, 8], f32)
        idxb = consts.tile([128, NPACKS, 8], u16)
        stage = consts.tile([128, NPACKS, 3], f16)
        id_sb = consts.tile([128, 128], f16)
        mk_sb = consts.tile([128, PACK], f16)
        scratch = consts.tile([128, 128], f16)
        # slots padded to 128 cols so multi-slot DMA runs are 512B-aligned;
        # two separate tiles so the first ship has no hazard with later copies
        ztail_a = consts.tile([128, len(TAIL_A), 128], f16)
        ztail_b = consts.tile([128, len(TAIL_B), 128], f16)
        # pad columns are shipped but never read by the host; memset them up
        # front (DVE is idle) so the interpreter sees initialized memory
        nc.vector.memset(ztail_a[:, :, PACK:128], 0.0)
        nc.vector.memset(ztail_b[:, :, PACK:128], 0.0)

        # Generate the 128x128 identity and the band mask on DVE (idle at
        # start) instead of spending DMA-stream bytes on them.
        # ident[p, f] = 1.0 iff p == f:  iota = p - f, is_equal 0.
        nc.gpsimd.memset(scratch[:], 1.0)
        nc.gpsimd.affine_select(
            out=id_sb[:], in_=scratch[:], pattern=[[-1, 128]],
            compare_op=mybir.AluOpType.is_equal, fill=0.0,
            base=0, channel_multiplier=1,
        )
        # maskT[w, (g, j)] = 0.0 iff 0 <= w - 9j < 9 else NEG (w = partition)
        nc.gpsimd.memset(scratch[:], 0.0)
        mk3 = mk_sb.rearrange("p (g j) -> p g j", j=14)
        nc.gpsimd.affine_select(
            out=mk3, in_=scratch[:, 0:PACK].rearrange("p (g j) -> p g j", j=14),
            pattern=[[0, 9], [-9, 14]],
            compare_op=mybir.AluOpType.is_ge, fill=NEG,
            base=0, channel_multiplier=1,
        )
        nc.gpsimd.affine_select(
            out=mk3, in_=mk3, pattern=[[0, 9], [9, 14]],
            compare_op=mybir.AluOpType.is_ge, fill=NEG,
            base=8, channel_multiplier=-1,
        )

        dma(out=hp_sb[:], in_=hp_d)
        dma(out=m_sb[:], in_=cst_d.rearrange("p (h c) -> p h c", c=C))

        mp_t = [None] * len(MP_TILES)
        for t in MP_ISSUE:
            first, _, w = MP_TILES[t]
            s0 = 1134 * first
            mt = mpp.tile([128, 2, w], f8, name=f"mp{t}", tag="mp_t")
            dma(out=mt[:], in_=mp_d[:, :, s0:s0 + w])
            mp_t[t] = mt

        psQ = ctx.enter_context(tc.tile_pool(name="psQ", bufs=2, space="PSUM"))
        psA = ctx.enter_context(tc.tile_pool(name="psA", bufs=3, space="PSUM"))
        psB = ctx.enter_context(tc.tile_pool(name="psB", bufs=3, space="PSUM"))

        def qm_block(ls):
            # qmT[c2, l] = sum_c1 M[c1, c2] * hpT[c1, l] for one 512-l block.
            # Copies stay on ACT: an in-order DVE queue would head-of-line
            # block the next segment's copy behind tile-gated pack maxes.
            for cc2 in range(2):
                qp = psQ.tile([128, 512], f32, tag="qp")
                for c1h in range(2):
                    nc.tensor.matmul(
                        qp[:],
                        m_sb[:, c1h, cc2 * 128:(cc2 + 1) * 128],
                        hp_sb[:, c1h, ls * 512:(ls + 1) * 512],
                        start=(c1h == 0),
                        stop=(c1h == 1),
                    )
                nc.scalar.copy(out=qmT_sb[:, cc2, ls * 512:(ls + 1) * 512],
                               in_=qp[:])

        def epilogue(lo, hi):
            """Ship raw argmax position + top-2 values for packs [lo, hi);
            the index -> (dy, dx) decode is pure formatting, done on host."""
            sl = slice(lo, hi)
            p_ = slice(0, PACK)
            nc.vector.tensor_copy(out=stage[p_, sl, 0], in_=idxb[p_, sl, 0])
            nc.vector.tensor_copy(out=stage[p_, sl, 1:3], in_=m8buf[p_, sl, 0:2])
            dma(out=out_d[0:PACK, sl, :], in_=stage[p_, sl, :])

        # Main loop: one pack of 126 pixels (32 for the tail) per iteration,
        # interleaved with the qmT block feeding the next segment.
        def pack_mm(k, pool=None):
            npx = PACK if k < NPACKS - 1 else L_SH - PACK * (NPACKS - 1)
            t_idx = k // 2 if k < 14 else (7 if k == 14 else 8)
            loc = 1134 * (k - MP_TILES[t_idx][0])
            # the last two tail packs borrow psQ's banks (idle after the
            # final qm block) so their matmuls never wait on psA recycling
            za = (pool or psA).tile([128, PACK], f32, tag="qp" if pool else "za",
                                    name="za")
            # Band mask lands first (opens the accumulation group): pixel
            # column m gets 0 at rows 9*(m%14)..+9, NEG elsewhere.
            nc.tensor.matmul(
                za[:],
                id_sb,
                mk_sb,
                start=True,
                stop=False,
                skip_group_check=True,
            )
            j0 = 0
            while j0 < npx:
                pg = min(14, npx - j0)
                for cc in range(2):
                    last = (j0 + pg >= npx) and cc == 1
                    nc.tensor.matmul(
                        za[0:9 * pg, j0:j0 + pg],
                        mp_t[t_idx][:, cc, loc + 9 * j0:loc + 9 * (j0 + pg)],
                        qmT_sb[:, cc, PACK * k + j0:PACK * k + j0 + pg],
                        start=False,
                        stop=last,
                        skip_group_check=True,
                    )
                j0 += pg
            return za

        def pack_fin(k, za):
            zt = ztp.tile([128, PACK], f16, tag="zt")
            nc.scalar.copy(out=zt[:], in_=za[:])
            zb = psB.tile([PACK, 128], f16, tag="zb")
            nc.tensor.transpose(zb[:], zt[:], id_sb)
            nc.vector.max(m8buf[0:PACK, k, :], zb[:])
            nc.vector.max_index(idxb[0:PACK, k, :], m8buf[0:PACK, k, :], zb[:])

        for blk, packs in QM_SEGMENTS:
            qm_block(blk)
            for k in packs:
                pack_fin(k, pack_mm(k))
        qm_block(3)
        epilogue(0, 12)
        # Tail: the last five packs skip the transpose/max path entirely --
        # their masked attn tiles ship raw (fp16) and the host takes the
        # 9-value band argmax.  Matmuls are hoisted ahead of the copies so
        # the PE queue never stalls behind an ACT copy, and pack 14 (whose
        # mp tile is the last DMA) rides alone in the final tiny ship.
        def fin_tail(tile_sb, slot, za, eng="act"):
            if eng == "act":
                nc.scalar.copy(out=tile_sb[:, slot, 0:PACK], in_=za[:])
            else:
                nc.vector.tensor_copy(out=tile_sb[:, slot, 0:PACK], in_=za[:])

        za_t = {k: pack_mm(k) for k in [12, 13, 15]}
        za_t[16] = pack_mm(16, pool=psQ)
        za14 = pack_mm(14, pool=psQ)
        fin_tail(ztail_a, 0, za_t[12])
        fin_tail(ztail_a, 1, za_t[13])
        fin_tail(ztail_a, 2, za_t[15], eng="dve")
        fin_tail(ztail_a, 3, za_t[16])
        dma(out=zta_d, in_=ztail_a[:])
        fin_tail(ztail_b, 0, za14)
        dma(out=ztb_d, in_=ztail_b[:])


def _get_nc():
    if "nc" not in _CACHE:
        _CACHE["nc"] = _build_nc()
    return _CACHE["nc"]


def make_in_maps(HSI_Patch, MSI_Patch2, W_qkv1, W_qkv2):
    import ml_dtypes

    f8 = ml_dtypes.float8_e4m3fn
    hp = np.asarray(HSI_Patch, np.float32).reshape(B, C, L)
    mp = np.asarray(MSI_Patch2, np.float32).reshape(B, C, 9 * L)

    # host-folded M = Wq @ Wk^T in float64, shipped fp16 in the
    # (c1%128, c1//128, c2) partition layout
    M = (np.asarray(W_qkv1, np.float64)[:, :C]
         @ np.asarray(W_qkv2, np.float64)[:, C:2 * C].T)
    cst = np.ascontiguousarray(
        M.reshape(2, 128, C).transpose(1, 0, 2).reshape(128, 2 * C)
    ).astype(np.float16)

    in_maps = []
    for core in range(8):
        b, half = core // 2, core % 2
        hp_sh = hp[b, :, half * L_SH:(half + 1) * L_SH]
        mp_sh = mp[b, :, half * S_SH:(half + 1) * S_SH]
        in_maps.append({
            "mp": np.ascontiguousarray(
                mp_sh.reshape(2, 128, S_SH).transpose(1, 0, 2)).astype(f8),
            "hp": np.ascontiguousarray(
                hp_sh.reshape(2, 128, L_SH).transpose(1, 0, 2)).astype(f8),
            "cst": cst,
        })
    return in_maps


def decode_shard(r, zt):
    """Raw device outputs -> (L_SH, 2) offsets + (L_SH,) gap.

    Packs 0..11 come decoded from r (128, NPACKS, 3): column 0 is the argmax
    position w in the transposed pack row; the band for pixel partition p
    sits at 9*(p%14), so n = w - 9*(p%14) in 0..8.  Tail packs 12..16
    arrive as raw masked attn tiles zt (128, 5, PACK) in fp16 -- the same
    values the device max would have scanned -- and the 9-value band argmax
    runs here instead.
    """
    out = np.zeros((L_SH, 2), np.float32)
    gap = np.zeros(L_SH, np.float32)
    base = 9 * (np.arange(128) % 14)
    for k in range(12):
        n = r[0:PACK, k, 0].astype(np.int32) - base[0:PACK]
        lo = PACK * k
        out[lo:lo + PACK, 0] = n // 3 - 1
        out[lo:lo + PACK, 1] = n % 3 - 1
        gap[lo:lo + PACK] = r[0:PACK, k, 1] - r[0:PACK, k, 2]
    m = np.arange(PACK)
    rows = (9 * (m % 14))[:, None] + np.arange(9)[None, :]    # (PACK, 9)
    for slot, k in [(i, k) for i, k in enumerate(TAIL_A)] + \
                   [(len(TAIL_A) + i, k) for i, k in enumerate(TAIL_B)]:
        npx = PACK if k < NPACKS - 1 else L_SH - PACK * (NPACKS - 1)
        vals = zt[rows[:npx], slot, m[:npx, None]].astype(np.float32)  # (npx, 9)
        srt = np.sort(vals, axis=1)
        n = vals.argmax(1)
        lo = PACK * k
        out[lo:lo + npx, 0] = n // 3 - 1
        out[lo:lo + npx, 1] = n % 3 - 1
        gap[lo:lo + npx] = srt[:, -1] - srt[:, -2]
    return out, gap


def gather_out(results):
    out = np.zeros((B, L, 2), np.float32)
    gap = np.zeros((B, L), np.float32)
    for core in range(8):
        b, half = core // 2, core % 2
        r = np.asarray(results[core]["outo"], np.float32)   # (128, 17, 3)
        za = np.asarray(results[core]["ztaila"]).reshape(128, len(TAIL_A), 128)
        zb = np.asarray(results[core]["ztailb"]).reshape(128, len(TAIL_B), 128)
        zt = np.concatenate([za, zb], axis=1)[:, :, 0:PACK]
        o, g = decode_shard(r, zt)
        out[b, half * L_SH:(half + 1) * L_SH] = o
        gap[b, half * L_SH:(half + 1) * L_SH] = g
    return out, gap


# Pixels whose top-2 attention gap is below this get an exact float64
# re-resolve on the host.  fp8 quantization of mp and hp each contribute
# ~0.06 of attn noise; the top-2 gap noise is ~0.118, so 0.5 is a ~4.2
# sigma guard band.
GAP_TAU = 0.5


def refine_ties(out, gap, HSI_Patch, MSI_Patch2, W_qkv1, W_qkv2):
    risky = np.argwhere(gap < GAP_TAU)
    if risky.size == 0:
        return out
    hp = np.asarray(HSI_Patch, np.float64).reshape(B, C, L)
    mp = np.asarray(MSI_Patch2, np.float64).reshape(B, C, 9 * L)
    Wq = np.asarray(W_qkv1, np.float64)[:, :C]
    Wk = np.asarray(W_qkv2, np.float64)[:, C:2 * C]
    M = Wq @ Wk.T
    rb, rl = risky[:, 0], risky[:, 1]
    qm = np.einsum("rc,cd->rd", hp[rb, :, rl], M)            # (R, C)
    win = (9 * rl)[:, None] + np.arange(9)[None, :]          # (R, 9)
    k9 = mp[rb[:, None], :, win]                             # (R, 9, C)
    n = np.einsum("rnc,rc->rn", k9, qm).argmax(1)
    out[rb, rl, 0] = n // 3 - 1
    out[rb, rl, 1] = n % 3 - 1
    return out


def kernel(x, y, HSI_Patch, MSI_Patch2, W_qkv1, W_qkv2, **_unused):
    import time

    from concourse.bass_utils import run_bass_kernel_spmd

    nc = _get_nc()
    in_maps = make_in_maps(HSI_Patch, MSI_Patch2, W_qkv1, W_qkv2)
    # A freshly-acquired NeuronCore occasionally reports a transient
    # NRT_EXEC_UNIT_UNRECOVERABLE from a previous tenant's aborted run;
    # a retry after a short pause recovers it.
    last_exc = None
    for attempt in range(3):
        try:
            res = run_bass_kernel_spmd(nc, in_maps, core_ids=list(range(8)))
            break
        except Exception as e:  # noqa: BLE001 -- retry only transient NRT states
            last_exc = e
            if "UNRECOVERABLE" not in str(e) and "UNAVAILABLE" not in str(e):
                raise
            time.sleep(5 * (attempt + 1))
    else:
        raise last_exc
    out, gap = gather_out(res.results)
    out = refine_ties(out, gap, HSI_Patch, MSI_Patch2, W_qkv1, W_qkv2)
    return out.reshape(B, H, H, 2)
